# revision 1
# baseline (speedup 1.0000x reference)
"""ConvSTFT on Trainium2: strided conv of x[32, 480000] against a fixed
[514, 1, 400] Fourier basis, hop 100 -> out [32, 514, 4803] f32.

Sharding: pure data parallel. Batch dim (32) split 4-per-core across 8
NeuronCores; the small weight is replicated.

Per-core device kernel (Bass/Tile):
  t = 100j + r decomposition (j in 0..3, r in 0..99) turns the overlapped
  conv into 4 PSUM-accumulated matmuls over a chunk-transposed view of x:
      out[c, f] = sum_j sum_r wT[100j + r, c] * XT[r, f + j]
  where XT[r, f'] = x_padded[100 f' + r].
  - x is loaded chunk-major ([128 chunks, 100]) and transposed on the PE
    (identity matmul) into XT[100, n_chunks] bf16 in SBUF.
  - matmuls: lhsT = wT[r, j, c-tile] (K=100, M<=128), rhs = XT[r, f-tile]
    (N<=512), fp32 PSUM accumulation over j.
  - PSUM evacuated by DVE/ACT into an SBUF row [<=128, 4803] f32, then one
    large contiguous DMA per (batch, channel-tile) to DRAM.
"""

import numpy as np
import ml_dtypes

WIN, HOP, C = 400, 100, 514
B, T = 32, 480000
PAD = WIN - HOP                       # 300
N_CORES = 8
B_LOC = B // N_CORES                  # 4
T_PAD = T + 2 * PAD                   # 480600
N_FRAMES = (T_PAD - WIN) // HOP + 1   # 4803
S_BLOCKS = -(-(T_PAD // HOP) // 128)  # 38 blocks of 128 chunks
N_CHUNKS = S_BLOCKS * 128             # 4864
T_DEV = N_CHUNKS * HOP                # 486400
NJ = WIN // HOP                       # 4

F_TILE = 512
C_TILE = 128


def build_program(b_loc=B_LOC, s_blocks=S_BLOCKS, n_frames=N_FRAMES):
    import concourse.bacc as bacc
    import concourse.mybir as mybir
    import concourse.tile as tile
    from concourse import masks

    dt = mybir.dt
    n_chunks = s_blocks * 128
    assert n_frames + NJ - 1 <= n_chunks

    nc = bacc.Bacc("TRN2", target_bir_lowering=False, debug=False)
    x_d = nc.dram_tensor(
        "x", [b_loc, n_chunks * HOP], dt.bfloat16, kind="ExternalInput"
    ).ap()
    w_d = nc.dram_tensor("wt", [WIN, C], dt.bfloat16, kind="ExternalInput").ap()
    o_d = nc.dram_tensor(
        "out", [b_loc, C, n_frames], dt.float32, kind="ExternalOutput"
    ).ap()

    ctiles = [(c0, min(C_TILE, C - c0)) for c0 in range(0, C, C_TILE)]
    ftiles = [(f0, min(F_TILE, n_frames - f0)) for f0 in range(0, n_frames, F_TILE)]

    with tile.TileContext(nc) as tc:
        with (
            tc.tile_pool(name="const", bufs=1) as constp,
            tc.tile_pool(name="nat", bufs=2) as natp,
            tc.tile_pool(name="xt", bufs=2) as xtp,
            tc.tile_pool(name="orow", bufs=3) as orowp,
            tc.tile_pool(name="mmps", bufs=5, space="PSUM") as mmps,
            tc.tile_pool(name="trps", bufs=3, space="PSUM") as trps,
        ):
            ident = constp.tile([128, 128], dt.bfloat16)
            masks.make_identity(nc, ident[:])
            wsb = constp.tile([HOP, NJ, C], dt.bfloat16)
            nc.sync.dma_start(wsb[:], w_d.rearrange("(j r) c -> r j c", r=HOP))

            ncopy = 0
            for b in range(b_loc):
                # chunk-major natural layout: nat[p, s, r] = x[b, (128 s + p) * 100 + r]
                nat = natp.tile([128, s_blocks, HOP], dt.bfloat16)
                nc.sync.dma_start(
                    nat[:], x_d[b].rearrange("(s p r) -> p s r", p=128, r=HOP)
                )
                # XT[r, f'] = x[b, 100 f' + r]
                xt = xtp.tile([128, n_chunks], dt.bfloat16)
                for g0 in range(0, s_blocks, 4):
                    gsz = min(4, s_blocks - g0)
                    tps = trps.tile([128, 512], dt.bfloat16)
                    for k in range(gsz):
                        nc.tensor.transpose(
                            tps[0:HOP, k * 128 : (k + 1) * 128],
                            nat[:, g0 + k, :],
                            ident[:],
                        )
                    nc.vector.tensor_copy(
                        xt[0:HOP, g0 * 128 : (g0 + gsz) * 128],
                        tps[0:HOP, 0 : gsz * 128],
                    )

                for c0, cm in ctiles:
                    orow = orowp.tile([128, n_frames], dt.float32)
                    for f0, fn in ftiles:
                        ps = mmps.tile([128, F_TILE], dt.float32)
                        for j in range(NJ):
                            nc.tensor.matmul(
                                ps[0:cm, 0:fn],
                                wsb[0:HOP, j, c0 : c0 + cm],
                                xt[0:HOP, f0 + j : f0 + j + fn],
                                start=(j == 0),
                                stop=(j == NJ - 1),
                            )
                        # split evacuation 2:1 between DVE and ACT
                        if ncopy % 3 == 2:
                            nc.scalar.copy(orow[0:cm, f0 : f0 + fn], ps[0:cm, 0:fn])
                        else:
                            nc.vector.tensor_copy(
                                orow[0:cm, f0 : f0 + fn], ps[0:cm, 0:fn]
                            )
                        ncopy += 1
                    nc.sync.dma_start(o_d[b, c0 : c0 + cm, :], orow[0:cm, :])

    nc.compile()
    return nc


_NC = None
LAST_RESULTS = None


def _prep_inputs(x, weight):
    x = np.asarray(x, dtype=np.float32)
    w = np.asarray(weight, dtype=np.float32)
    xp = np.zeros((x.shape[0], T_DEV), dtype=np.float32)
    xp[:, PAD : PAD + T] = x
    xbf = xp.astype(ml_dtypes.bfloat16)
    wt = np.ascontiguousarray(w.reshape(C, WIN).T).astype(ml_dtypes.bfloat16)
    return xbf, wt


def kernel(x, weight):
    global _NC, LAST_RESULTS
    from concourse.bass_utils import run_bass_kernel_spmd

    xbf, wt = _prep_inputs(x, weight)
    if _NC is None:
        _NC = build_program()
    in_maps = [
        {"x": np.ascontiguousarray(xbf[c * B_LOC : (c + 1) * B_LOC]), "wt": wt}
        for c in range(N_CORES)
    ]
    res = run_bass_kernel_spmd(_NC, in_maps, core_ids=list(range(N_CORES)))
    LAST_RESULTS = res
    out = np.concatenate([r["out"] for r in res.results], axis=0)
    return np.ascontiguousarray(out)


# revision 2
# speedup vs baseline: 1.0761x; 1.0761x over previous
"""ConvSTFT on Trainium2: strided conv of x[32, 480000] against a fixed
[514, 1, 400] Fourier basis, hop 100 -> out [32, 514, 4803] f32.

Sharding: pure data parallel. Batch dim (32) split 4-per-core across 8
NeuronCores; the small weight is replicated.

Host prep (sharding layer): pad x by 300 on both sides, then lay it out
chunk-transposed in blocks of 128 hops:
    x_dev[b, s, r, p] = x_padded[b, (128 s + p) * 100 + r]
so the device can DMA straight into XT[r, f'] = x_padded[100 f' + r]
(f' = 128 s + p) with 256-byte contiguous lines. The weight is passed
transposed: wt[t, c] = weight[c, 0, t]. Both are cast to bf16.

Per-core device kernel (Bass/Tile):
  t = 100j + r decomposition (j in 0..3, r in 0..99) turns the overlapped
  conv into 4 PSUM-accumulated matmuls:
      out[c, f] = sum_j sum_r wt[100j + r, c] * XT[r, f + j]
  - lhsT = wt[r, j, c-tile] (K=100, M<=128), rhs = XT[r, f-tile] (N<=512),
    fp32 PSUM accumulation over j, all 8 PSUM banks in flight.
  - PSUM evacuated alternately by DVE/ACT into an SBUF row [<=128, 4803]
    f32, stored with two large contiguous DMAs per (batch, channel-tile).
This streams the PE at its floor (1 bf16 column/cycle; 20 tile-streams
per frame-column = ceil(514/128) * ceil(400/128) is provably minimal).
"""

import numpy as np
import ml_dtypes

WIN, HOP, C = 400, 100, 514
B, T = 32, 480000
PAD = WIN - HOP                       # 300
N_CORES = 8
B_LOC = B // N_CORES                  # 4
T_PAD = T + 2 * PAD                   # 480600
N_FRAMES = (T_PAD - WIN) // HOP + 1   # 4803
S_BLOCKS = -(-(T_PAD // HOP) // 128)  # 38 blocks of 128 chunks
N_CHUNKS = S_BLOCKS * 128             # 4864
NJ = WIN // HOP                       # 4

F_TILE = 512
C_TILE = 128
LOAD_GRP = 8                          # s-blocks per input DMA piece
STORE_SPLIT = 5                       # store first half after this many ftiles


def build_program(b_loc=B_LOC, s_blocks=S_BLOCKS, n_frames=N_FRAMES):
    import concourse.bacc as bacc
    import concourse.mybir as mybir
    import concourse.tile as tile

    dt = mybir.dt
    n_chunks = s_blocks * 128
    assert n_frames + NJ - 1 <= n_chunks

    nc = bacc.Bacc("TRN2", target_bir_lowering=False, debug=False)
    x_d = nc.dram_tensor(
        "x", [b_loc, s_blocks, HOP, 128], dt.bfloat16, kind="ExternalInput"
    ).ap()
    w_d = nc.dram_tensor("wt", [WIN, C], dt.bfloat16, kind="ExternalInput").ap()
    o_d = nc.dram_tensor(
        "out", [b_loc, C, n_frames], dt.float32, kind="ExternalOutput"
    ).ap()

    ctiles = [(c0, min(C_TILE, C - c0)) for c0 in range(0, C, C_TILE)]
    ftiles = [(f0, min(F_TILE, n_frames - f0)) for f0 in range(0, n_frames, F_TILE)]

    with tile.TileContext(nc) as tc:
        with (
            tc.tile_pool(name="const", bufs=1) as constp,
            tc.tile_pool(name="xt", bufs=2) as xtp,
            tc.tile_pool(name="orow", bufs=3) as orowp,
            tc.tile_pool(name="mmps", bufs=8, space="PSUM") as mmps,
        ):
            wsb = constp.tile([HOP, NJ, C], dt.bfloat16)
            nc.scalar.dma_start(wsb[:], w_d.rearrange("(j r) c -> r j c", r=HOP))

            ncopy = 0
            for b in range(b_loc):
                # XT[r, f'] = x_padded[b, 100 f' + r], loaded in pieces so
                # matmuls can start as soon as the first columns land.
                xt = xtp.tile([HOP, s_blocks, 128], dt.bfloat16)
                for g0 in range(0, s_blocks, LOAD_GRP):
                    gs = min(LOAD_GRP, s_blocks - g0)
                    nc.scalar.dma_start(
                        xt[:, g0 : g0 + gs, :],
                        x_d[b, g0 : g0 + gs].rearrange("g r p -> r g p"),
                    )
                xtf = xt.rearrange("r g p -> r (g p)")

                for c0, cm in ctiles:
                    orow = orowp.tile([128, n_frames], dt.float32)
                    for fi, (f0, fn) in enumerate(ftiles):
                        ps = mmps.tile([128, F_TILE], dt.float32)
                        for j in range(NJ):
                            nc.tensor.matmul(
                                ps[0:cm, 0:fn],
                                wsb[0:HOP, j, c0 : c0 + cm],
                                xtf[0:HOP, f0 + j : f0 + j + fn],
                                start=(j == 0),
                                stop=(j == NJ - 1),
                            )
                        # alternate evacuation between DVE and ACT
                        if ncopy % 2 == 1:
                            nc.scalar.copy(orow[0:cm, f0 : f0 + fn], ps[0:cm, 0:fn])
                        else:
                            nc.vector.tensor_copy(
                                orow[0:cm, f0 : f0 + fn], ps[0:cm, 0:fn]
                            )
                        ncopy += 1
                        if fi == STORE_SPLIT - 1 and len(ftiles) > STORE_SPLIT:
                            mid = ftiles[STORE_SPLIT][0]
                            nc.sync.dma_start(
                                o_d[b, c0 : c0 + cm, 0:mid], orow[0:cm, 0:mid]
                            )
                    mid = (
                        ftiles[STORE_SPLIT][0]
                        if len(ftiles) > STORE_SPLIT
                        else 0
                    )
                    nc.sync.dma_start(
                        o_d[b, c0 : c0 + cm, mid:n_frames],
                        orow[0:cm, mid:n_frames],
                    )

    nc.compile()
    return nc


_NC = None
LAST_RESULTS = None


def _prep_inputs(x, weight):
    x = np.asarray(x, dtype=np.float32)
    w = np.asarray(weight, dtype=np.float32)
    nb = x.shape[0]
    xp = np.zeros((nb, N_CHUNKS * HOP), dtype=np.float32)
    xp[:, PAD : PAD + x.shape[1]] = x
    # chunk-block mini-transpose: [b, s, p, r] -> [b, s, r, p]
    xdev = np.ascontiguousarray(
        xp.reshape(nb, S_BLOCKS, 128, HOP).transpose(0, 1, 3, 2)
    ).astype(ml_dtypes.bfloat16)
    wt = np.ascontiguousarray(w.reshape(C, WIN).T).astype(ml_dtypes.bfloat16)
    return xdev, wt


def kernel(x, weight):
    global _NC, LAST_RESULTS
    from concourse.bass_utils import run_bass_kernel_spmd

    xdev, wt = _prep_inputs(x, weight)
    if _NC is None:
        _NC = build_program()
    in_maps = [
        {"x": np.ascontiguousarray(xdev[c * B_LOC : (c + 1) * B_LOC]), "wt": wt}
        for c in range(N_CORES)
    ]
    res = run_bass_kernel_spmd(_NC, in_maps, core_ids=list(range(N_CORES)))
    LAST_RESULTS = res
    out = np.concatenate([r["out"] for r in res.results], axis=0)
    return np.ascontiguousarray(out)
